# revision 7
# baseline (speedup 1.0000x reference)
"""EvolveGCN (GRU-evolved GCN layer + linear head) on 8 Trainium2 NeuronCores.

Strategy: shard destination nodes (and their incoming edges) across 8 cores
(49 tiles of 128 dest nodes per core). Per dest tile the kernel batch-gathers
the edges' source rows of x (bf16) with the custom DMAGatherAnt instruction
(queue-rotated across the 4 SWDGE queues so descriptor generation runs on all
8 GPSIMD Q7 cores), builds a norm-scaled one-hot matrix on DVE, and
accumulates h_pre^T = sum_e x[src_e]^T (x) onehot(dst_e)*norm_e in PSUM via
TensorE (using (A X) W == A (X W)). Self-loops use a contiguous per-core x
slice and a scaled-identity matmul instead of gathers. Epilogue per tile
(evolved-W matmul, relu, linear head) runs in f32. The small GRU weight
evolution runs replicated on every core.
"""

import os
import sys

import numpy as np

try:
    import concourse.bass as bass
except ImportError:  # fresh grading dir: fall back to the staged repo
    for p in ("/opt/trn_rl_repo", "/root/.axon_site/_ro/trn_rl_repo"):
        if os.path.isdir(p) and p not in sys.path:
            sys.path.insert(0, p)
    import concourse.bass as bass

import ml_dtypes
import concourse.mybir as mybir
import concourse.tile as tile
from concourse import bacc
from concourse.bass_utils import run_bass_kernel_spmd

P = 128
N_NODES = 50000
F_DIM = 128
N_TARGET = 8
NCORES = 8
TILES_PER_CORE = 49            # 49 * 128 = 6272; 8 * 6272 = 50176 >= 50000
ROWS_PER_CORE = TILES_PER_CORE * P
NT_GLOBAL = NCORES * TILES_PER_CORE  # 392 padded global tiles
SPLIT = 32768                  # x row split so gather indices fit int16
NLO = SPLIT
NHI = N_NODES - SPLIT          # 17232

_COMPILED = {}
_LAST_RESULTS = None


def _host_prep(edge_index, edge_weight):
    """Sort/pad non-self-loop edges by (dest tile, src half); per-core metas.

    Returns (idx16_maps, cn_maps, sdiag, clo, chi) where per tile the first
    clo*128 chunk slots gather from x[:SPLIT] and the next chi*128 from
    x[SPLIT:].
    """
    row = np.asarray(edge_index[0], dtype=np.int64)
    col = np.asarray(edge_index[1], dtype=np.int64)
    ew = np.asarray(edge_weight, dtype=np.float64)

    deg = np.bincount(col, weights=ew, minlength=N_NODES) + 1.0
    dinv = 1.0 / np.sqrt(deg)

    norm = (dinv[row] * ew * dinv[col]).astype(np.float32)
    tile_id = col >> 7
    half = (row >= SPLIT).astype(np.int64)   # 0 = lo, 1 = hi
    group = tile_id * 2 + half
    order = np.argsort(group, kind="stable")
    g_sorted = group[order]
    cnt = np.bincount(g_sorted, minlength=NT_GLOBAL * 2)
    chunks = (cnt + P - 1) // P
    clo = int(chunks[0::2].max())
    chi = int(chunks[1::2].max())

    starts = np.zeros(NT_GLOBAL * 2, dtype=np.int64)
    starts[1:] = np.cumsum(cnt)[:-1]
    pos = np.arange(g_sorted.size) - starts[g_sorted]
    # slot base per group: tile*(clo+chi)*128, hi groups offset by clo*128
    t_of_g = np.arange(NT_GLOBAL * 2) // 2
    base_of_g = t_of_g * ((clo + chi) * P) + (np.arange(NT_GLOBAL * 2) % 2) * (clo * P)
    slots = base_of_g[g_sorted] + pos

    tot = NT_GLOBAL * (clo + chi) * P
    idx_flat = np.zeros(tot, dtype=np.int16)
    colrel_flat = np.zeros(tot, dtype=np.float32)
    norm_flat = np.zeros(tot, dtype=np.float32)
    src_sorted = row[order]
    idx_flat[slots] = (src_sorted - half[order] * SPLIT).astype(np.int16)
    colrel_flat[slots] = (col[order] & 127).astype(np.float32)
    norm_flat[slots] = norm[order]

    nch = TILES_PER_CORE * (clo + chi)
    idx_core = idx_flat.reshape(NCORES, nch, P)
    colrel_core = colrel_flat.reshape(NCORES, nch, P).astype(ml_dtypes.bfloat16)
    norm_core = norm_flat.reshape(NCORES, nch, P).astype(ml_dtypes.bfloat16)

    idx16_maps = []
    cn_maps = []
    cnact_maps = []
    for c in range(NCORES):
        # wrap for dma_gather: per instruction (tile-pair, half) block of
        # NIDX idxs -> [16, NIDX/16] (idx i at [i%16, i//16]), replicated x8.
        per_tile = idx_core[c].reshape(TILES_PER_CORE, (clo + chi) * P)
        blocks = []
        for t0 in range(0, TILES_PER_CORE, 2):
            ts = [t0] if t0 + 1 >= TILES_PER_CORE else [t0, t0 + 1]
            lo = np.concatenate([per_tile[t, :clo * P] for t in ts])
            hi = np.concatenate([per_tile[t, clo * P:] for t in ts])
            blocks.append(np.concatenate(
                [lo.reshape(-1, 16).T, hi.reshape(-1, 16).T], axis=1))
        blk = np.concatenate(blocks, axis=1)          # [16, nch*8]
        idx16_maps.append(np.ascontiguousarray(np.tile(blk, (8, 1))))
        ncr = colrel_core[c].T.astype(np.float32)
        nno = norm_core[c].T.astype(np.float32)
        cn_maps.append(np.ascontiguousarray(np.concatenate(
            [ncr, nno], axis=1).astype(ml_dtypes.bfloat16)))
        cnact_maps.append(np.ascontiguousarray(
            np.concatenate([-ncr, -nno, nno], axis=1).astype(np.float32)))

    # self-loop scale dinv^2 per dest row, [128, 49] per core (bf16)
    s = (dinv * dinv).astype(np.float32)
    s_pad = np.zeros(NT_GLOBAL * P, dtype=np.float32)
    s_pad[:N_NODES] = s
    sdiag = s_pad.reshape(NCORES, TILES_PER_CORE, P)
    sdiag_maps = [np.ascontiguousarray(sdiag[c].T.astype(ml_dtypes.bfloat16))
                  for c in range(NCORES)]
    return idx16_maps, cn_maps, cnact_maps, sdiag_maps, clo, chi


def _build_program(clo, chi):
    nch = TILES_PER_CORE * (clo + chi)
    dt = mybir.dt

    nc = bacc.Bacc("TRN2", target_bir_lowering=False, debug=False,
                   num_devices=NCORES, num_swdge_queues=4)

    xlo_d = nc.declare_dram_parameter("xlo", [NLO, F_DIM], dt.bfloat16,
                                      isOutput=False)
    xhi_d = nc.declare_dram_parameter("xhi", [NHI, F_DIM], dt.bfloat16,
                                      isOutput=False)
    xself_d = nc.declare_dram_parameter("xself", [ROWS_PER_CORE, F_DIM],
                                        dt.bfloat16, isOutput=False)
    idx_d = nc.declare_dram_parameter("idx16", [P, nch * 8], dt.int16,
                                      isOutput=False)
    cn_d = nc.declare_dram_parameter("cn", [P, 2 * nch], dt.bfloat16,
                                     isOutput=False)
    cnact_d = nc.declare_dram_parameter("cnact", [P, 3 * nch], dt.float32,
                                        isOutput=False)
    sdiag_d = nc.declare_dram_parameter("sdiag", [P, TILES_PER_CORE],
                                        dt.bfloat16, isOutput=False)
    w0t_d = nc.declare_dram_parameter("w0t", [F_DIM, F_DIM], dt.float32,
                                      isOutput=False)
    wiht_d = nc.declare_dram_parameter("wiht", [F_DIM, 3 * F_DIM], dt.float32,
                                       isOutput=False)
    whht_d = nc.declare_dram_parameter("whht", [F_DIM, 3 * F_DIM], dt.float32,
                                       isOutput=False)
    bias4_d = nc.declare_dram_parameter("bias4", [F_DIM, 4], dt.float32,
                                        isOutput=False)
    wlint_d = nc.declare_dram_parameter("wlint", [F_DIM, N_TARGET], dt.float32,
                                        isOutput=False)
    blin_d = nc.declare_dram_parameter("blin", [P, N_TARGET], dt.float32,
                                       isOutput=False)
    out_d = nc.declare_dram_parameter("out", [ROWS_PER_CORE, N_TARGET],
                                      dt.float32, isOutput=True)

    with tile.TileContext(nc) as tc:
        with (
            tc.tile_pool(name="const", bufs=1) as cpool,
            tc.tile_pool(name="gath", bufs=3) as gpool,
            tc.tile_pool(name="work", bufs=8) as wpool,
            tc.tile_pool(name="epi", bufs=3) as epool,
            tc.tile_pool(name="psum", bufs=2, space="PSUM") as ppool,
        ):
            # ---- constants ----
            iota_i = cpool.tile([P, P], dtype=dt.int32)
            nc.gpsimd.iota(iota_i[:], pattern=[[1, P]], base=0,
                           channel_multiplier=0)
            iota_bf = cpool.tile([P, P], dtype=dt.bfloat16)
            nc.vector.tensor_copy(iota_bf[:], iota_i[:])
            # identity in bf16: (iota[d] == partition_idx)
            pidx_i = cpool.tile([P, 1], dtype=dt.int32)
            nc.gpsimd.iota(pidx_i[:], pattern=[[0, 1]], base=0,
                           channel_multiplier=1)
            pidx_bf = cpool.tile([P, 1], dtype=dt.bfloat16)
            nc.vector.tensor_copy(pidx_bf[:], pidx_i[:])
            ident_bf = cpool.tile([P, P], dtype=dt.bfloat16)
            nc.vector.tensor_tensor(out=ident_bf[:], in0=iota_bf[:],
                                    in1=pidx_bf[:].to_broadcast([P, P]),
                                    op=mybir.AluOpType.is_equal)

            idx_sb = cpool.tile([P, nch * 8], dtype=dt.int16)
            cn_sb = cpool.tile([P, 2 * nch], dtype=dt.bfloat16)
            cnact_sb = cpool.tile([P, 3 * nch], dtype=dt.float32)
            nc.sync.dma_start(out=cnact_sb[:], in_=cnact_d[:])
            sdiag_sb = cpool.tile([P, TILES_PER_CORE], dtype=dt.bfloat16)
            nc.sync.dma_start(out=idx_sb[:], in_=idx_d[:])
            nc.sync.dma_start(out=cn_sb[:], in_=cn_d[:])
            nc.sync.dma_start(out=sdiag_sb[:], in_=sdiag_d[:])

            w0t_sb = cpool.tile([P, F_DIM], dtype=dt.float32)
            wiht_sb = cpool.tile([P, 3 * F_DIM], dtype=dt.float32)
            whht_sb = cpool.tile([P, 3 * F_DIM], dtype=dt.float32)
            bias4_sb = cpool.tile([P, 4], dtype=dt.float32)
            wlint_sb = cpool.tile([P, N_TARGET], dtype=dt.float32)
            blin_sb = cpool.tile([P, N_TARGET], dtype=dt.float32)
            nc.sync.dma_start(out=w0t_sb[:], in_=w0t_d[:])
            nc.sync.dma_start(out=wiht_sb[:], in_=wiht_d[:])
            nc.sync.dma_start(out=whht_sb[:], in_=whht_d[:])
            nc.sync.dma_start(out=bias4_sb[:], in_=bias4_d[:])
            nc.sync.dma_start(out=wlint_sb[:], in_=wlint_d[:])
            nc.sync.dma_start(out=blin_sb[:], in_=blin_d[:])

            # ---- GRU weight evolution (transposed gates: [j, k]) ----
            sig = mybir.ActivationFunctionType.Sigmoid
            tanh = mybir.ActivationFunctionType.Tanh
            ident_f = mybir.ActivationFunctionType.Identity

            ps_r = ppool.tile([P, P], dtype=dt.float32, space="PSUM", tag="hpre")
            nc.tensor.matmul(out=ps_r[:], lhsT=wiht_sb[:, 0:128],
                             rhs=w0t_sb[:], start=True, stop=False)
            nc.tensor.matmul(out=ps_r[:], lhsT=whht_sb[:, 0:128],
                             rhs=w0t_sb[:], start=False, stop=True)
            rT = cpool.tile([P, P], dtype=dt.float32, tag="gru_rT")
            nc.scalar.activation(rT[:], ps_r[:], sig, bias=bias4_sb[:, 0:1])

            ps_z = ppool.tile([P, P], dtype=dt.float32, space="PSUM", tag="hpre")
            nc.tensor.matmul(out=ps_z[:], lhsT=wiht_sb[:, 128:256],
                             rhs=w0t_sb[:], start=True, stop=False)
            nc.tensor.matmul(out=ps_z[:], lhsT=whht_sb[:, 128:256],
                             rhs=w0t_sb[:], start=False, stop=True)
            zT = cpool.tile([P, P], dtype=dt.float32, tag="gru_zT")
            nc.scalar.activation(zT[:], ps_z[:], sig, bias=bias4_sb[:, 1:2])

            ps_in = ppool.tile([P, P], dtype=dt.float32, space="PSUM", tag="hpre")
            nc.tensor.matmul(out=ps_in[:], lhsT=wiht_sb[:, 256:384],
                             rhs=w0t_sb[:], start=True, stop=True)
            ps_hn = ppool.tile([P, P], dtype=dt.float32, space="PSUM", tag="hpre")
            nc.tensor.matmul(out=ps_hn[:], lhsT=whht_sb[:, 256:384],
                             rhs=w0t_sb[:], start=True, stop=True)
            hnT = cpool.tile([P, P], dtype=dt.float32, tag="gru_hnT")
            nc.scalar.activation(hnT[:], ps_hn[:], ident_f,
                                 bias=bias4_sb[:, 3:4])
            t1 = cpool.tile([P, P], dtype=dt.float32, tag="gru_t1")
            nc.vector.tensor_tensor(out=t1[:], in0=rT[:], in1=hnT[:],
                                    op=mybir.AluOpType.mult)
            nc.vector.tensor_tensor(out=t1[:], in0=t1[:], in1=ps_in[:],
                                    op=mybir.AluOpType.add)
            nT = cpool.tile([P, P], dtype=dt.float32, tag="gru_nT")
            nc.scalar.activation(nT[:], t1[:], tanh, bias=bias4_sb[:, 2:3])
            # W^T = n^T + z^T * (W0^T - n^T)
            t3 = cpool.tile([P, P], dtype=dt.float32, tag="gru_t3")
            nc.vector.tensor_tensor(out=t3[:], in0=w0t_sb[:], in1=nT[:],
                                    op=mybir.AluOpType.subtract)
            nc.vector.tensor_tensor(out=t3[:], in0=zT[:], in1=t3[:],
                                    op=mybir.AluOpType.mult)
            wT_sb = cpool.tile([P, P], dtype=dt.float32)
            nc.vector.tensor_tensor(out=wT_sb[:], in0=nT[:], in1=t3[:],
                                    op=mybir.AluOpType.add)
            # W [k, j] = transpose(W^T) via PE transpose (needs f32 identity)
            ident_f32 = cpool.tile([P, P], dtype=dt.float32)
            nc.vector.tensor_copy(ident_f32[:], ident_bf[:])
            ps_w = ppool.tile([P, P], dtype=dt.float32, space="PSUM", tag="hpre")
            nc.tensor.transpose(out=ps_w[:], in_=wT_sb[:], identity=ident_f32[:])
            w_sb = cpool.tile([P, P], dtype=dt.float32)
            nc.scalar.copy(w_sb[:], ps_w[:])

            # ---- main loop over dest tile pairs ----
            qrot = 0
            for t0 in range(0, TILES_PER_CORE, 2):
                npair = 1 if t0 + 1 >= TILES_PER_CORE else 2
                # paired gathers: lo and hi halves, queue-rotated
                xg_lo = gpool.tile([P, 2 * clo * F_DIM], dtype=dt.bfloat16,
                                   tag="xg_lo")
                xg_hi = gpool.tile([P, 2 * chi * F_DIM], dtype=dt.bfloat16,
                                   tag="xg_hi")
                ibase = t0 * (clo + chi) * 8
                nlo = npair * clo * P
                nhi = npair * chi * P
                nc.gpsimd.dma_gather(
                    xg_lo[:, :npair * clo * F_DIM].rearrange(
                        "p (c f) -> p c f", f=F_DIM),
                    xlo_d[:],
                    idx_sb[:, ibase:ibase + nlo // 16],
                    nlo, nlo, F_DIM,
                    single_packet=False, queue_num=qrot % 4)
                qrot += 1
                nc.gpsimd.dma_gather(
                    xg_hi[:, :npair * chi * F_DIM].rearrange(
                        "p (c f) -> p c f", f=F_DIM),
                    xhi_d[:],
                    idx_sb[:, ibase + nlo // 16:ibase + (nlo + nhi) // 16],
                    nhi, nhi, F_DIM,
                    single_packet=False, queue_num=qrot % 4)
                qrot += 1

                for dt_i in range(npair):
                    t = t0 + dt_i
                    hpreT_ps = ppool.tile([P, P], dtype=dt.float32,
                                          space="PSUM", tag="hpre")
                    # self-loop: X_self_tile^T @ diag(s)
                    xs = wpool.tile([P, F_DIM], dtype=dt.bfloat16, tag="xs")
                    nc.sync.dma_start(out=xs[:],
                                      in_=xself_d[t * P:(t + 1) * P, :])
                    sdg = wpool.tile([P, P], dtype=dt.bfloat16, tag="sdg")
                    nc.vector.tensor_tensor(
                        out=sdg[:], in0=ident_bf[:],
                        in1=sdiag_sb[:, t:t + 1].to_broadcast([P, P]),
                        op=mybir.AluOpType.mult)
                    nc.tensor.matmul(out=hpreT_ps[:], lhsT=xs[:], rhs=sdg[:],
                                     start=True, stop=False)

                    for ci in range(clo + chi):
                        j = t * (clo + chi) + ci
                        xg = (xg_lo if ci < clo else xg_hi)
                        cin = (ci if ci < clo else ci - clo)
                        coff = (cin + dt_i * (clo if ci < clo else chi)) * F_DIM
                        a_mat = wpool.tile([P, P], dtype=dt.bfloat16, tag="a")
                        if ci % 4 == 3:
                            # ACT path: t_ = |iota - colrel|; a = relu(norm - t_*norm)
                            t_abs = wpool.tile([P, P], dtype=dt.bfloat16,
                                               tag="tabs")
                            nc.scalar.activation(
                                t_abs[:], iota_bf[:],
                                mybir.ActivationFunctionType.Abs,
                                bias=cnact_sb[:, j:j + 1])
                            nc.scalar.activation(
                                a_mat[:], t_abs[:],
                                mybir.ActivationFunctionType.Relu,
                                scale=cnact_sb[:, nch + j:nch + j + 1],
                                bias=cnact_sb[:, 2 * nch + j:2 * nch + j + 1])
                        else:
                            nc.vector.scalar_tensor_tensor(
                                out=a_mat[:], in0=iota_bf[:],
                                scalar=cn_sb[:, j:j + 1],
                                in1=cn_sb[:, nch + j:nch + j + 1].to_broadcast(
                                    [P, P]),
                                op0=mybir.AluOpType.is_equal,
                                op1=mybir.AluOpType.mult,
                            )
                        nc.tensor.matmul(out=hpreT_ps[:],
                                         lhsT=xg[:, coff:coff + F_DIM],
                                         rhs=a_mat[:],
                                         start=False,
                                         stop=(ci == clo + chi - 1))

                    # epilogue: h^T = W-matmul; relu; head matmul; +bias
                    hpreT_sb = epool.tile([P, P], dtype=dt.float32,
                                          tag="hpre_sb")
                    nc.scalar.copy(hpreT_sb[:], hpreT_ps[:])
                    hT_ps = ppool.tile([P, P], dtype=dt.float32, space="PSUM",
                                       tag="ht")
                    nc.tensor.matmul(out=hT_ps[:], lhsT=w_sb[:],
                                     rhs=hpreT_sb[:], start=True, stop=True)
                    hT_relu = epool.tile([P, P], dtype=dt.float32, tag="ht_sb")
                    nc.scalar.activation(hT_relu[:], hT_ps[:],
                                         mybir.ActivationFunctionType.Relu)
                    out_ps = ppool.tile([P, N_TARGET], dtype=dt.float32,
                                        space="PSUM", tag="out")
                    nc.tensor.matmul(out=out_ps[:], lhsT=hT_relu[:],
                                     rhs=wlint_sb[:], start=True, stop=True)
                    out_sb = epool.tile([P, N_TARGET], dtype=dt.float32,
                                        tag="out_sb")
                    nc.vector.tensor_tensor(out=out_sb[:], in0=out_ps[:],
                                            in1=blin_sb[:],
                                            op=mybir.AluOpType.add)
                    nc.sync.dma_start(out=out_d[t * P:(t + 1) * P, :],
                                      in_=out_sb[:])

    nc.compile()
    return nc


def kernel(x, edge_index, edge_weight, W0, Wih, Whh, bih, bhh, Wlin, blin):
    x = np.ascontiguousarray(np.asarray(x, dtype=np.float32))
    (idx16_maps, cn_maps, cnact_maps, sdiag_maps, clo,
     chi) = _host_prep(edge_index, edge_weight)

    key = (clo, chi)
    if key not in _COMPILED:
        _COMPILED[key] = _build_program(clo, chi)
    nc = _COMPILED[key]

    x_bf = x.astype(ml_dtypes.bfloat16)
    xlo = np.ascontiguousarray(x_bf[:SPLIT])
    xhi = np.ascontiguousarray(x_bf[SPLIT:])
    xself_pad = np.zeros((NT_GLOBAL * P, F_DIM), dtype=ml_dtypes.bfloat16)
    xself_pad[:N_NODES] = x_bf

    W0 = np.asarray(W0, dtype=np.float32)
    Wih = np.asarray(Wih, dtype=np.float32)
    Whh = np.asarray(Whh, dtype=np.float32)
    bih = np.asarray(bih, dtype=np.float32)
    bhh = np.asarray(bhh, dtype=np.float32)
    Wlin = np.asarray(Wlin, dtype=np.float32)
    blin = np.asarray(blin, dtype=np.float32)

    w0t = np.ascontiguousarray(W0.T)
    wiht = np.ascontiguousarray(Wih.T)   # [F, 3F]
    whht = np.ascontiguousarray(Whh.T)
    bias4 = np.stack(
        [bih[0:128] + bhh[0:128], bih[128:256] + bhh[128:256],
         bih[256:384], bhh[256:384]], axis=1,
    ).astype(np.float32)                  # [128, 4]
    wlint = np.ascontiguousarray(Wlin.T)  # [F, 8]
    blin_rep = np.ascontiguousarray(np.tile(blin[None, :], (P, 1)))

    in_maps = []
    for c in range(NCORES):
        in_maps.append({
            "xlo": xlo, "xhi": xhi,
            "xself": np.ascontiguousarray(
                xself_pad[c * ROWS_PER_CORE:(c + 1) * ROWS_PER_CORE]),
            "idx16": idx16_maps[c], "cn": cn_maps[c],
            "cnact": cnact_maps[c], "sdiag": sdiag_maps[c],
            "w0t": w0t, "wiht": wiht, "whht": whht, "bias4": bias4,
            "wlint": wlint, "blin": blin_rep,
        })

    trace = os.environ.get("GCN_TRACE", "0") == "1"
    res = run_bass_kernel_spmd(
        nc, in_maps, list(range(NCORES)), trace=trace,
        trace_cores=list(range(NCORES)) if trace else None,
    )
    global _LAST_RESULTS
    _LAST_RESULTS = res
    if trace and res.exec_time_ns is not None:
        print(f"HW exec time: {res.exec_time_ns} ns")

    parts = []
    for c in range(NCORES):
        rows = min(ROWS_PER_CORE, N_NODES - c * ROWS_PER_CORE)
        parts.append(res.results[c]["out"][:rows])
    return np.concatenate(parts, axis=0)


# revision 9
# speedup vs baseline: 1.6183x; 1.6183x over previous
"""EvolveGCN (GRU-evolved GCN layer + linear head) on 8 Trainium2 NeuronCores.

Strategy: shard destination nodes (and their incoming edges) across 8 cores
(49 tiles of 128 dest nodes per core). Per dest tile the kernel batch-gathers
the edges' source rows of x (bf16) with the custom DMAGatherAnt instruction
(queue-rotated across the 4 SWDGE queues so descriptor generation runs on all
8 GPSIMD Q7 cores), builds a norm-scaled one-hot matrix on DVE, and
accumulates h_pre^T = sum_e x[src_e]^T (x) onehot(dst_e)*norm_e in PSUM via
TensorE (using (A X) W == A (X W)). Self-loops use a contiguous per-core x
slice and a scaled-identity matmul instead of gathers. Epilogue per tile
(evolved-W matmul, relu, linear head) runs in f32. The small GRU weight
evolution runs replicated on every core.
"""

import os
import sys

import numpy as np

try:
    import concourse.bass as bass
except ImportError:  # fresh grading dir: fall back to the staged repo
    for p in ("/opt/trn_rl_repo", "/root/.axon_site/_ro/trn_rl_repo"):
        if os.path.isdir(p) and p not in sys.path:
            sys.path.insert(0, p)
    import concourse.bass as bass

import ml_dtypes
import concourse.mybir as mybir
import concourse.tile as tile
from concourse import bacc
from concourse.bass_utils import run_bass_kernel_spmd

P = 128
N_NODES = 50000
F_DIM = 128
N_TARGET = 8
NCORES = 8
TILES_PER_CORE = 49            # 49 * 128 = 6272; 8 * 6272 = 50176 >= 50000
ROWS_PER_CORE = TILES_PER_CORE * P
NT_GLOBAL = NCORES * TILES_PER_CORE  # 392 padded global tiles
SPLIT = 32768                  # x row split so gather indices fit int16
NLO = SPLIT
NHI = N_NODES - SPLIT          # 17232

_COMPILED = {}
_LAST_RESULTS = None


def _host_prep(edge_index, edge_weight):
    """Sort/pad non-self-loop edges by (dest tile, src half); per-core metas.

    Tiles within each core are processed in descending edge-count order
    (per-core permutation, host unpermutes outputs); chunk counts per
    position are the max across cores, which keeps padding small while the
    SPMD program stays identical on all cores.
    """
    row = np.asarray(edge_index[0], dtype=np.int64)
    col = np.asarray(edge_index[1], dtype=np.int64)
    ew = np.asarray(edge_weight, dtype=np.float64)

    deg = np.bincount(col, weights=ew, minlength=N_NODES) + 1.0
    dinv = 1.0 / np.sqrt(deg)

    norm = (dinv[row] * ew * dinv[col]).astype(np.float32)
    tile_id = col >> 7
    half = (row >= SPLIT).astype(np.int64)   # 0 = lo, 1 = hi
    group = tile_id * 2 + half
    order = np.argsort(group, kind="stable")
    g_sorted = group[order]
    cnt = np.bincount(g_sorted, minlength=NT_GLOBAL * 2)
    lo_cnt = cnt[0::2].reshape(NCORES, TILES_PER_CORE)
    hi_cnt = cnt[1::2].reshape(NCORES, TILES_PER_CORE)
    tot_cnt = lo_cnt + hi_cnt

    # per-core tile order: descending total count
    perms = np.argsort(-tot_cnt, axis=1, kind="stable")   # [NCORES, T]
    lo_sorted = np.take_along_axis(lo_cnt, perms, axis=1)
    hi_sorted = np.take_along_axis(hi_cnt, perms, axis=1)
    clop = tuple(int(v) for v in
                 np.ceil(lo_sorted.max(axis=0) / P).astype(np.int64))
    chip = tuple(int(v) for v in
                 np.ceil(hi_sorted.max(axis=0) / P).astype(np.int64))

    # slot base for each (core, position, half)
    per_pos = np.array(clop) + np.array(chip)           # chunks per position
    pos_base = np.zeros(TILES_PER_CORE, dtype=np.int64)
    pos_base[1:] = np.cumsum(per_pos)[:-1]              # chunk offset
    nch = int(per_pos.sum())

    # group (c, tloc, half) -> slot base (within core, in edge slots)
    inv = np.empty_like(perms)
    for c in range(NCORES):
        inv[c, perms[c]] = np.arange(TILES_PER_CORE)
    g_core = np.arange(NT_GLOBAL * 2) // (2 * TILES_PER_CORE)
    g_tloc = (np.arange(NT_GLOBAL * 2) // 2) % TILES_PER_CORE
    g_half = np.arange(NT_GLOBAL * 2) % 2
    g_pos = inv[g_core, g_tloc]
    base_of_g = (g_core * nch + pos_base[g_pos]
                 + g_half * np.array(clop)[g_pos]) * P

    starts = np.zeros(NT_GLOBAL * 2, dtype=np.int64)
    starts[1:] = np.cumsum(cnt)[:-1]
    pos_in = np.arange(g_sorted.size) - starts[g_sorted]
    slots = base_of_g[g_sorted] + pos_in

    tot = NCORES * nch * P
    idx_flat = np.zeros(tot, dtype=np.int16)
    colrel_flat = np.zeros(tot, dtype=np.float32)
    norm_flat = np.zeros(tot, dtype=np.float32)
    idx_flat[slots] = (row[order] - half[order] * SPLIT).astype(np.int16)
    colrel_flat[slots] = (col[order] & 127).astype(np.float32)
    norm_flat[slots] = norm[order]

    idx_core = idx_flat.reshape(NCORES, nch, P)
    colrel_core = colrel_flat.reshape(NCORES, nch, P)
    norm_core = norm_flat.reshape(NCORES, nch, P)

    idx16_maps = []
    cn_maps = []
    for c in range(NCORES):
        per_core = idx_core[c].reshape(nch * P)
        blocks = []
        for j in range(TILES_PER_CORE):
            b0 = pos_base[j] * P
            lo = per_core[b0:b0 + clop[j] * P].reshape(-1, 16).T
            hi = per_core[b0 + clop[j] * P:b0 + per_pos[j] * P].reshape(-1, 16).T
            blocks.append(np.concatenate([lo, hi], axis=1))
        blk = np.concatenate(blocks, axis=1)          # [16, nch*8]
        idx16_maps.append(np.ascontiguousarray(np.tile(blk, (8, 1))))
        cn_maps.append(np.ascontiguousarray(
            np.concatenate([colrel_core[c].T, norm_core[c].T],
                           axis=1).astype(ml_dtypes.bfloat16)))

    # self-loop scale dinv^2 per dest row, position-ordered per core
    s = (dinv * dinv).astype(np.float32)
    s_pad = np.zeros(NT_GLOBAL * P, dtype=np.float32)
    s_pad[:N_NODES] = s
    sdiag = s_pad.reshape(NCORES, TILES_PER_CORE, P)
    sdiag_maps = [np.ascontiguousarray(
        sdiag[c][perms[c]].T.astype(ml_dtypes.bfloat16)) for c in range(NCORES)]
    return idx16_maps, cn_maps, sdiag_maps, clop, chip, perms


def _build_program(clop, chip):
    per_pos = [a + b for a, b in zip(clop, chip)]
    pos_base = [0] * TILES_PER_CORE
    for j in range(1, TILES_PER_CORE):
        pos_base[j] = pos_base[j - 1] + per_pos[j - 1]
    nch = sum(per_pos)
    dt = mybir.dt

    nc = bacc.Bacc("TRN2", target_bir_lowering=False, debug=False,
                   num_devices=NCORES, num_swdge_queues=4)

    xlo_d = nc.declare_dram_parameter("xlo", [NLO, F_DIM], dt.bfloat16,
                                      isOutput=False)
    xhi_d = nc.declare_dram_parameter("xhi", [NHI, F_DIM], dt.bfloat16,
                                      isOutput=False)
    xself_d = nc.declare_dram_parameter("xself", [ROWS_PER_CORE, F_DIM],
                                        dt.bfloat16, isOutput=False)
    idx_d = nc.declare_dram_parameter("idx16", [P, nch * 8], dt.int16,
                                      isOutput=False)
    cn_d = nc.declare_dram_parameter("cn", [P, 2 * nch], dt.bfloat16,
                                     isOutput=False)
    sdiag_d = nc.declare_dram_parameter("sdiag", [P, TILES_PER_CORE],
                                        dt.bfloat16, isOutput=False)
    w0t_d = nc.declare_dram_parameter("w0t", [F_DIM, F_DIM], dt.float32,
                                      isOutput=False)
    wiht_d = nc.declare_dram_parameter("wiht", [F_DIM, 3 * F_DIM], dt.float32,
                                       isOutput=False)
    whht_d = nc.declare_dram_parameter("whht", [F_DIM, 3 * F_DIM], dt.float32,
                                       isOutput=False)
    bias4_d = nc.declare_dram_parameter("bias4", [F_DIM, 4], dt.float32,
                                        isOutput=False)
    wlint_d = nc.declare_dram_parameter("wlint", [F_DIM, N_TARGET], dt.float32,
                                        isOutput=False)
    blin_d = nc.declare_dram_parameter("blin", [P, N_TARGET], dt.float32,
                                       isOutput=False)
    out_d = nc.declare_dram_parameter("out", [ROWS_PER_CORE, N_TARGET],
                                      dt.float32, isOutput=True)

    with tile.TileContext(nc) as tc:
        with (
            tc.tile_pool(name="const", bufs=1) as cpool,
            tc.tile_pool(name="gath", bufs=3) as gpool,
            tc.tile_pool(name="work", bufs=8) as wpool,
            tc.tile_pool(name="epi", bufs=3) as epool,
            tc.tile_pool(name="psum", bufs=2, space="PSUM") as ppool,
        ):
            # ---- constants ----
            iota_i = cpool.tile([P, P], dtype=dt.int32)
            nc.gpsimd.iota(iota_i[:], pattern=[[1, P]], base=0,
                           channel_multiplier=0)
            iota_bf = cpool.tile([P, P], dtype=dt.bfloat16)
            nc.vector.tensor_copy(iota_bf[:], iota_i[:])
            # identity in bf16: (iota[d] == partition_idx)
            pidx_i = cpool.tile([P, 1], dtype=dt.int32)
            nc.gpsimd.iota(pidx_i[:], pattern=[[0, 1]], base=0,
                           channel_multiplier=1)
            pidx_bf = cpool.tile([P, 1], dtype=dt.bfloat16)
            nc.vector.tensor_copy(pidx_bf[:], pidx_i[:])
            ident_bf = cpool.tile([P, P], dtype=dt.bfloat16)
            nc.vector.tensor_tensor(out=ident_bf[:], in0=iota_bf[:],
                                    in1=pidx_bf[:].to_broadcast([P, P]),
                                    op=mybir.AluOpType.is_equal)

            idx_sb = cpool.tile([P, nch * 8], dtype=dt.int16)
            cn_sb = cpool.tile([P, 2 * nch], dtype=dt.bfloat16)
            sdiag_sb = cpool.tile([P, TILES_PER_CORE], dtype=dt.bfloat16)
            nc.sync.dma_start(out=idx_sb[:], in_=idx_d[:])
            nc.sync.dma_start(out=cn_sb[:], in_=cn_d[:])
            nc.sync.dma_start(out=sdiag_sb[:], in_=sdiag_d[:])

            w0t_sb = cpool.tile([P, F_DIM], dtype=dt.float32)
            wiht_sb = cpool.tile([P, 3 * F_DIM], dtype=dt.float32)
            whht_sb = cpool.tile([P, 3 * F_DIM], dtype=dt.float32)
            bias4_sb = cpool.tile([P, 4], dtype=dt.float32)
            wlint_sb = cpool.tile([P, N_TARGET], dtype=dt.float32)
            blin_sb = cpool.tile([P, N_TARGET], dtype=dt.float32)
            nc.sync.dma_start(out=w0t_sb[:], in_=w0t_d[:])
            nc.sync.dma_start(out=wiht_sb[:], in_=wiht_d[:])
            nc.sync.dma_start(out=whht_sb[:], in_=whht_d[:])
            nc.sync.dma_start(out=bias4_sb[:], in_=bias4_d[:])
            nc.sync.dma_start(out=wlint_sb[:], in_=wlint_d[:])
            nc.sync.dma_start(out=blin_sb[:], in_=blin_d[:])

            # ---- GRU weight evolution (transposed gates: [j, k]) ----
            sig = mybir.ActivationFunctionType.Sigmoid
            tanh = mybir.ActivationFunctionType.Tanh
            ident_f = mybir.ActivationFunctionType.Identity

            ps_r = ppool.tile([P, P], dtype=dt.float32, space="PSUM", tag="hpre")
            nc.tensor.matmul(out=ps_r[:], lhsT=wiht_sb[:, 0:128],
                             rhs=w0t_sb[:], start=True, stop=False)
            nc.tensor.matmul(out=ps_r[:], lhsT=whht_sb[:, 0:128],
                             rhs=w0t_sb[:], start=False, stop=True)
            rT = cpool.tile([P, P], dtype=dt.float32, tag="gru_rT")
            nc.scalar.activation(rT[:], ps_r[:], sig, bias=bias4_sb[:, 0:1])

            ps_z = ppool.tile([P, P], dtype=dt.float32, space="PSUM", tag="hpre")
            nc.tensor.matmul(out=ps_z[:], lhsT=wiht_sb[:, 128:256],
                             rhs=w0t_sb[:], start=True, stop=False)
            nc.tensor.matmul(out=ps_z[:], lhsT=whht_sb[:, 128:256],
                             rhs=w0t_sb[:], start=False, stop=True)
            zT = cpool.tile([P, P], dtype=dt.float32, tag="gru_zT")
            nc.scalar.activation(zT[:], ps_z[:], sig, bias=bias4_sb[:, 1:2])

            ps_in = ppool.tile([P, P], dtype=dt.float32, space="PSUM", tag="hpre")
            nc.tensor.matmul(out=ps_in[:], lhsT=wiht_sb[:, 256:384],
                             rhs=w0t_sb[:], start=True, stop=True)
            ps_hn = ppool.tile([P, P], dtype=dt.float32, space="PSUM", tag="hpre")
            nc.tensor.matmul(out=ps_hn[:], lhsT=whht_sb[:, 256:384],
                             rhs=w0t_sb[:], start=True, stop=True)
            hnT = cpool.tile([P, P], dtype=dt.float32, tag="gru_hnT")
            nc.scalar.activation(hnT[:], ps_hn[:], ident_f,
                                 bias=bias4_sb[:, 3:4])
            t1 = cpool.tile([P, P], dtype=dt.float32, tag="gru_t1")
            nc.vector.tensor_tensor(out=t1[:], in0=rT[:], in1=hnT[:],
                                    op=mybir.AluOpType.mult)
            nc.vector.tensor_tensor(out=t1[:], in0=t1[:], in1=ps_in[:],
                                    op=mybir.AluOpType.add)
            nT = cpool.tile([P, P], dtype=dt.float32, tag="gru_nT")
            nc.scalar.activation(nT[:], t1[:], tanh, bias=bias4_sb[:, 2:3])
            # W^T = n^T + z^T * (W0^T - n^T)
            t3 = cpool.tile([P, P], dtype=dt.float32, tag="gru_t3")
            nc.vector.tensor_tensor(out=t3[:], in0=w0t_sb[:], in1=nT[:],
                                    op=mybir.AluOpType.subtract)
            nc.vector.tensor_tensor(out=t3[:], in0=zT[:], in1=t3[:],
                                    op=mybir.AluOpType.mult)
            wT_sb = cpool.tile([P, P], dtype=dt.float32)
            nc.vector.tensor_tensor(out=wT_sb[:], in0=nT[:], in1=t3[:],
                                    op=mybir.AluOpType.add)
            # W [k, j] = transpose(W^T) via PE transpose (needs f32 identity)
            ident_f32 = cpool.tile([P, P], dtype=dt.float32)
            nc.vector.tensor_copy(ident_f32[:], ident_bf[:])
            ps_w = ppool.tile([P, P], dtype=dt.float32, space="PSUM", tag="hpre")
            nc.tensor.transpose(out=ps_w[:], in_=wT_sb[:], identity=ident_f32[:])
            w_sb = cpool.tile([P, P], dtype=dt.float32)
            nc.scalar.copy(w_sb[:], ps_w[:])

            # ---- main loop over dest tile positions ----
            qrot = 0
            for t in range(TILES_PER_CORE):
                clo = clop[t]
                chi = chip[t]
                hpreT_ps = ppool.tile([P, P], dtype=dt.float32, space="PSUM",
                                      tag="hpre")
                # self-loop: X_self_tile^T @ diag(s)
                xs = wpool.tile([P, F_DIM], dtype=dt.bfloat16, tag="xs")
                nc.sync.dma_start(out=xs[:],
                                  in_=xself_d[t * P:(t + 1) * P, :])
                sdg = wpool.tile([P, P], dtype=dt.bfloat16, tag="sdg")
                nc.vector.tensor_tensor(
                    out=sdg[:], in0=ident_bf[:],
                    in1=sdiag_sb[:, t:t + 1].to_broadcast([P, P]),
                    op=mybir.AluOpType.mult)
                nc.tensor.matmul(out=hpreT_ps[:], lhsT=xs[:], rhs=sdg[:],
                                 start=True, stop=False)

                # gathers: lo and hi halves, queue-rotated
                xg_lo = gpool.tile([P, max(clop) * F_DIM], dtype=dt.bfloat16,
                                   tag="xg_lo")
                xg_hi = gpool.tile([P, max(max(chip), 1) * F_DIM],
                                   dtype=dt.bfloat16, tag="xg_hi")
                ibase = pos_base[t] * 8
                nc.gpsimd.dma_gather(
                    xg_lo[:, :clo * F_DIM].rearrange("p (c f) -> p c f",
                                                     f=F_DIM),
                    xlo_d[:],
                    idx_sb[:, ibase:ibase + clo * 8],
                    clo * P, clo * P, F_DIM,
                    single_packet=False, queue_num=qrot % 4)
                qrot += 1
                if chi > 0:
                    nc.gpsimd.dma_gather(
                        xg_hi[:, :chi * F_DIM].rearrange("p (c f) -> p c f",
                                                         f=F_DIM),
                        xhi_d[:],
                        idx_sb[:, ibase + clo * 8:ibase + (clo + chi) * 8],
                        chi * P, chi * P, F_DIM,
                        single_packet=False, queue_num=qrot % 4)
                    qrot += 1

                for ci in range(clo + chi):
                    j = pos_base[t] + ci
                    xg = (xg_lo if ci < clo else xg_hi)
                    coff = (ci if ci < clo else ci - clo) * F_DIM
                    a_mat = wpool.tile([P, P], dtype=dt.bfloat16, tag="a")
                    nc.vector.scalar_tensor_tensor(
                        out=a_mat[:], in0=iota_bf[:],
                        scalar=cn_sb[:, j:j + 1],
                        in1=cn_sb[:, nch + j:nch + j + 1].to_broadcast([P, P]),
                        op0=mybir.AluOpType.is_equal,
                        op1=mybir.AluOpType.mult,
                    )
                    nc.tensor.matmul(out=hpreT_ps[:],
                                     lhsT=xg[:, coff:coff + F_DIM],
                                     rhs=a_mat[:],
                                     start=False, stop=(ci == clo + chi - 1))

                # epilogue: h^T = W-matmul; relu; head matmul; +bias (f32)
                hpreT_sb = epool.tile([P, P], dtype=dt.float32, tag="hpre_sb")
                nc.scalar.copy(hpreT_sb[:], hpreT_ps[:])
                hT_ps = ppool.tile([P, P], dtype=dt.float32, space="PSUM",
                                   tag="ht")
                nc.tensor.matmul(out=hT_ps[:], lhsT=w_sb[:], rhs=hpreT_sb[:],
                                 start=True, stop=True)
                hT_relu = epool.tile([P, P], dtype=dt.float32, tag="ht_sb")
                nc.scalar.activation(hT_relu[:], hT_ps[:],
                                     mybir.ActivationFunctionType.Relu)
                out_ps = ppool.tile([P, N_TARGET], dtype=dt.float32,
                                    space="PSUM", tag="out")
                nc.tensor.matmul(out=out_ps[:], lhsT=hT_relu[:],
                                 rhs=wlint_sb[:], start=True, stop=True)
                out_sb = epool.tile([P, N_TARGET], dtype=dt.float32,
                                    tag="out_sb")
                nc.vector.tensor_tensor(out=out_sb[:], in0=out_ps[:],
                                        in1=blin_sb[:],
                                        op=mybir.AluOpType.add)
                nc.sync.dma_start(out=out_d[t * P:(t + 1) * P, :],
                                  in_=out_sb[:])

    nc.compile()
    return nc


def kernel(x, edge_index, edge_weight, W0, Wih, Whh, bih, bhh, Wlin, blin):
    x = np.ascontiguousarray(np.asarray(x, dtype=np.float32))
    idx16_maps, cn_maps, sdiag_maps, clop, chip, perms = _host_prep(
        edge_index, edge_weight)

    key = (clop, chip)
    if key not in _COMPILED:
        _COMPILED[key] = _build_program(clop, chip)
    nc = _COMPILED[key]

    x_bf = x.astype(ml_dtypes.bfloat16)
    xlo = np.ascontiguousarray(x_bf[:SPLIT])
    xhi = np.ascontiguousarray(x_bf[SPLIT:])
    xself_pad = np.zeros((NT_GLOBAL * P, F_DIM), dtype=ml_dtypes.bfloat16)
    xself_pad[:N_NODES] = x_bf

    W0 = np.asarray(W0, dtype=np.float32)
    Wih = np.asarray(Wih, dtype=np.float32)
    Whh = np.asarray(Whh, dtype=np.float32)
    bih = np.asarray(bih, dtype=np.float32)
    bhh = np.asarray(bhh, dtype=np.float32)
    Wlin = np.asarray(Wlin, dtype=np.float32)
    blin = np.asarray(blin, dtype=np.float32)

    w0t = np.ascontiguousarray(W0.T)
    wiht = np.ascontiguousarray(Wih.T)   # [F, 3F]
    whht = np.ascontiguousarray(Whh.T)
    bias4 = np.stack(
        [bih[0:128] + bhh[0:128], bih[128:256] + bhh[128:256],
         bih[256:384], bhh[256:384]], axis=1,
    ).astype(np.float32)                  # [128, 4]
    wlint = np.ascontiguousarray(Wlin.T)  # [F, 8]
    blin_rep = np.ascontiguousarray(np.tile(blin[None, :], (P, 1)))

    in_maps = []
    for c in range(NCORES):
        in_maps.append({
            "xlo": xlo, "xhi": xhi,
            "xself": np.ascontiguousarray(
                xself_pad[c * ROWS_PER_CORE:(c + 1) * ROWS_PER_CORE]
                .reshape(TILES_PER_CORE, P, F_DIM)[perms[c]]
                .reshape(ROWS_PER_CORE, F_DIM)),
            "idx16": idx16_maps[c], "cn": cn_maps[c], "sdiag": sdiag_maps[c],
            "w0t": w0t, "wiht": wiht, "whht": whht, "bias4": bias4,
            "wlint": wlint, "blin": blin_rep,
        })

    trace = os.environ.get("GCN_TRACE", "0") == "1"
    res = run_bass_kernel_spmd(
        nc, in_maps, list(range(NCORES)), trace=trace,
        trace_cores=list(range(NCORES)) if trace else None,
    )
    global _LAST_RESULTS
    _LAST_RESULTS = res
    if trace and res.exec_time_ns is not None:
        print(f"HW exec time: {res.exec_time_ns} ns")

    parts = []
    for c in range(NCORES):
        blk = res.results[c]["out"].reshape(TILES_PER_CORE, P, N_TARGET)
        inv = np.empty(TILES_PER_CORE, dtype=np.int64)
        inv[perms[c]] = np.arange(TILES_PER_CORE)
        unperm = blk[inv].reshape(ROWS_PER_CORE, N_TARGET)
        rows = min(ROWS_PER_CORE, N_NODES - c * ROWS_PER_CORE)
        parts.append(unperm[:rows])
    return np.concatenate(parts, axis=0)
